# revision 18
# baseline (speedup 1.0000x reference)
"""Fused LayerNorm + multi-head attention + output projection on 8 TRN2 cores.

Sharding: core c handles batch b = c//4 and head group g = c%4 (4 of 16 heads).
Each core computes LN(x[b]) (replicated within the batch's 4 cores), the qkv
projection for its heads, attention, and a partial output projection (w_out
rows for its heads). The host sums the 4 partials per batch.

On-chip layout is fully transposed ([feature, token]); the host pre-transposes
x, folds gamma / softmax scale / beta into the weights, and packs everything in
SBUF-ready layouts, so the kernel needs zero on-chip transposes:

  xn^T   [D, T]   = LayerNorm(x)^T        (stats via ones-matmul broadcast)
  q^T/k^T [dh, T] = W_q/k^T-slices @ xn^T (feature-major)
  v      [T, dh]  = xn^T-tiles.T @ W_v    (token-major, swapped operands)
  E^T    [k, q]   = exp(K Q^T)            (no max subtraction: scores ~N(0,1))
  av^T   [dh, q]  = V-block @ E^T         (denominator rows ride along in M)
  out^T  [D, T]   = w_out-slice^T @ (av^T/den)

V-block layout packs head pairs densely: even head -> [den@0 | 0*63 | V@64:128],
odd head -> [V@0:64 | zeros | den@96], so the attention output lands dense in a
2-chunk aot and the output projection contracts over 2 chunks instead of 4.
The odd head's denominator is hopped from partition 96 to 0 via a tiny DMA
before the partition broadcast (broadcast must source partition 0).

NOTE: keep the per-partition SBUF envelope under 208 KiB (asserted post-
compile) — allocations past it land in the bass reserve and corrupt
framework state at runtime in ways that show up as garbage tiles.
"""

import numpy as np

HEADS = 16
DIM_HEAD = 64
SCALE = DIM_HEAD**-0.5
EPS = 1e-5
B, S, D = 2, 2048, 1024
T = S
NCORES = 8
NH = 4  # heads per core
F = 3 * NH * DIM_HEAD  # 768 features per core: [q(256) | k(256) | v(256)]
DC = D // 128  # 8 contraction chunks
KC = T // 128  # 16 key chunks
QB = 4  # q blocks
QW = T // QB  # 512 q block width

_cache = {}


def _src_tag():
    # Cache-buster: the jit/NEFF cache can key on the HLO signature without
    # the embedded kernel program; giving the module an input whose shape
    # depends on this file's content forces a recompile when the kernel
    # changes.
    import hashlib
    with open(__file__, "rb") as f:
        return int(hashlib.sha256(f.read()).hexdigest()[:8], 16) % 120 + 8


def _build():
    import concourse.bacc as bacc
    import concourse.mybir as mybir
    import concourse.tile as tile

    fp32 = mybir.dt.float32
    bf16 = mybir.dt.bfloat16
    i16 = mybir.dt.int16
    AF = mybir.ActivationFunctionType
    ALU = mybir.AluOpType
    # Schraudolph exp offload to the DVE for these key-chunks of each block:
    # round(s*128/ln2 + B) as int16, bitcast bf16 == exp(s)*(1+-3%); the
    # softmax denominator mixes engines, so B centers the log error to keep
    # the approximation unbiased vs the exact ACT exps.
    SCHR_CHUNKS = _cache.get("schr_chunks", frozenset())
    SCHR_A = 128.0 / float(np.log(2.0))
    SCHR_B = 127.0 * 128.0 - 7.34

    nc = bacc.Bacc("TRN2", target_bir_lowering=False, debug=False,
                   num_devices=NCORES)
    xt_d = nc.declare_dram_parameter("xt", [128, DC * T], bf16, isOutput=False)
    wqkv_d = nc.declare_dram_parameter("wqkv", [128, DC * F], bf16, isOutput=False)
    wout_d = nc.declare_dram_parameter("wout", [128, 2 * D], bf16, isOutput=False)
    bqkv_d = nc.declare_dram_parameter("bqkv", [1, F], bf16, isOutput=False)
    bqkc_d = nc.declare_dram_parameter("bqkc", [128, 4], fp32, isOutput=False)
    bout_d = nc.declare_dram_parameter("bout", [128, 8], fp32, isOutput=False)
    seed_d = nc.declare_dram_parameter("seed", [1, _src_tag()], fp32,
                                       isOutput=False)
    out_d = nc.declare_dram_parameter("out", [D, T], fp32, isOutput=True)
    dbg = {}
    if _cache.get("debug"):
        dbg["xn"] = nc.declare_dram_parameter("dbg_xn", [128, DC * T], bf16, isOutput=True)
        dbg["qk"] = nc.declare_dram_parameter("dbg_qk", [128, 4 * T], bf16, isOutput=True)
        dbg["vsb"] = nc.declare_dram_parameter("dbg_vsb", [128, KC * NH * 128], bf16, isOutput=True)
        dbg["aot"] = nc.declare_dram_parameter("dbg_aot", [128, 2 * T], bf16, isOutput=True)

    with tile.TileContext(nc) as tc:
        with (
            tc.tile_pool(name="const", bufs=1) as constp,
            tc.tile_pool(name="big", bufs=1) as bigp,
            tc.tile_pool(name="work", bufs=2) as workp,
            tc.tile_pool(name="psum", bufs=1, space="PSUM") as psump,
        ):
            # ---- persistent SBUF ----
            seedt = constp.tile([1, 128], fp32, tag="seedt")
            nc.sync.dma_start(seedt[0:1, 0:seed_d.shape[1]], seed_d[:])
            ones128 = constp.tile([128, 128], bf16, tag="ones128")
            nc.gpsimd.memset(ones128[:], 1.0)
            onesrow = constp.tile([1, QW], bf16, tag="onesrow")
            nc.gpsimd.memset(onesrow[:], 1.0)
            wqkv = constp.tile([128, DC * F], bf16, tag="wqkv")
            wout = constp.tile([128, 2 * D], bf16, tag="wout")
            bqkv = constp.tile([1, F], bf16, tag="bqkv")
            bqkc = constp.tile([128, 4], fp32, tag="bqkc")
            bout = constp.tile([128, 8], fp32, tag="bout")

            xn = bigp.tile([128, DC * T], bf16, tag="xn")  # normalized x^T
            # q^T / k^T feature-major: m=0,1 -> q pairs 0,1; m=2,3 -> k
            qk = bigp.tile([128, 4 * T], bf16, tag="qk")
            # v blocks, 128 wide per (k-chunk, head):
            #   even head: [den(1) | zeros(63) | V(64)]  (V at rows 64:128)
            #   odd head:  [V(64) | zeros | den@96 | zeros]  (V at rows 0:64)
            vsb = bigp.tile([128, KC * NH * 128], bf16, tag="vsb")
            nc.gpsimd.memset(vsb[:], 0.0)
            vsb_rr = vsb[:].rearrange("p (c hp x) -> p c hp x", hp=2, x=256)
            nc.gpsimd.memset(vsb_rr[:, :, :, 0:1], 1.0)      # even-head den col
            nc.gpsimd.memset(vsb_rr[:, :, :, 224:225], 1.0)  # odd-head den col (row 96)
            # attention output^T, dense: chunk p = head pair p,
            # rows 64:128 = even head 2p, rows 0:64 = odd head 2p+1
            aot = bigp.tile([128, 2 * T], bf16, tag="aot")

            # psum slots: 4 tags x [128, 1024] (2 banks each) = 8 banks
            ps_n = [0]

            def ps(tag):
                ps_n[0] += 1
                return psump.tile([128, 1024], fp32, tag=tag,
                                  name=f"ps_{tag}_{ps_n[0]}")

            ab = [0]

            def ps_ab():
                ab[0] += 1
                return ps(["psA", "psB"][ab[0] % 2])

            def qk_mms(m, tbp, half, slot):
                tb = tbp * 2 + half
                o = slot[:, half * 512:(half + 1) * 512]
                for c in range(DC):
                    nc.tensor.matmul(
                        o,
                        wqkv[:, c * F + m * 128:c * F + (m + 1) * 128],
                        xn[:, c * T + tb * 512:c * T + (tb + 1) * 512],
                        start=(c == 0), stop=(c == DC - 1))

            def qk_bias(m, tbp, slot):
                nc.vector.tensor_scalar(
                    out=qk[:, m * T + tbp * 1024:m * T + (tbp + 1) * 1024],
                    in0=slot[:], scalar1=bqkc[:, m:m + 1], scalar2=None,
                    op0=ALU.add)

            def qk_proj_part(m, tbp):
                slot = ps_ab()
                qk_mms(m, tbp, 0, slot)
                qk_mms(m, tbp, 1, slot)
                qk_bias(m, tbp, slot)

            # ================= Phase 1: LayerNorm =================
            with (tc.tile_pool(name="ln", bufs=1) as lnp,
                  tc.tile_pool(name="lnw", bufs=2) as lnwp):
                xt = lnp.tile([128, DC * T], bf16, tag="xt")
                mean_b = lnp.tile([128, T], bf16, tag="mean_b")
                rstd_b = lnp.tile([128, T], bf16, tag="rstd_b")
                # x first: it gates everything. Weights after (needed later).
                for c in range(DC):
                    csl = slice(c * T, (c + 1) * T)
                    nc.sync.dma_start(xt[:, csl], xt_d[:, csl])
                nc.sync.dma_start(bqkc[:], bqkc_d[:])
                nc.sync.dma_start(wqkv[:], wqkv_d[:])
                nc.sync.dma_start(bqkv[:], bqkv_d[:])
                nc.sync.dma_start(bout[:], bout_d[:])
                nc.sync.dma_start(wout[:], wout_d[:])
                x2 = xn  # scratch: squares are consumed before xn is written
                for c in range(DC):
                    csl = slice(c * T, (c + 1) * T)
                    nc.vector.tensor_tensor(out=x2[:, csl], in0=xt[:, csl],
                                            in1=xt[:, csl], op=ALU.mult)
                # stats, chunk-outer so matmuls chase the DMAs; the x^2 pass
                # lags one chunk so the DVE stays ahead of the PE
                slots = [ps(t) for t in ("psA", "psB", "psC", "psD")]
                for c in range(DC + 1):
                    if c < DC:
                        for tb in range(4):
                            sl = slice(c * T + tb * 512, c * T + (tb + 1) * 512)
                            nc.tensor.matmul(slots[tb][:, 0:512], ones128[:],
                                             xt[:, sl],
                                             start=(c == 0), stop=(c == DC - 1))
                    if c > 0:
                        cm = c - 1
                        for tb in range(4):
                            sl = slice(cm * T + tb * 512, cm * T + (tb + 1) * 512)
                            nc.tensor.matmul(slots[tb][:, 512:1024], ones128[:],
                                             x2[:, sl],
                                             start=(cm == 0), stop=(cm == DC - 1))
                for tb in range(4):
                    s_ps, q_ps = slots[tb][:, 0:512], slots[tb][:, 512:1024]
                    tsl = slice(tb * 512, (tb + 1) * 512)
                    nc.vector.tensor_scalar(out=mean_b[:, tsl], in0=s_ps,
                                            scalar1=1.0 / D, scalar2=None,
                                            op0=ALU.mult)
                    t1 = lnwp.tile([128, 512], fp32, tag="lnt1")
                    nc.vector.tensor_scalar(out=t1[:], in0=q_ps,
                                            scalar1=1.0 / D, scalar2=EPS,
                                            op0=ALU.mult, op1=ALU.add)
                    m2 = lnwp.tile([128, 512], fp32, tag="lnm2")
                    nc.vector.tensor_tensor(out=m2[:], in0=mean_b[:, tsl],
                                            in1=mean_b[:, tsl], op=ALU.mult)
                    var = lnwp.tile([128, 512], fp32, tag="lnvar")
                    nc.vector.tensor_tensor(out=var[:], in0=t1[:], in1=m2[:],
                                            op=ALU.subtract)
                    rcv = lnwp.tile([128, 512], fp32, tag="lnrcv")
                    nc.vector.reciprocal_approx_fast(out=rcv[:], in_=var[:])
                    nc.scalar.activation(rstd_b[:, tsl], rcv[:], AF.Sqrt)
                # preload the exp table set while ACT is otherwise idle, so the
                # first real exp doesn't pay the ~1.3us table load
                dume = lnwp.tile([1, 8], fp32, tag="dume")
                nc.scalar.activation(dume[:], rstd_b[0:1, 0:8], AF.Exp)
                # normalize in half-token panels: q/k projections for the first
                # token half can start while the second half is still on the DVE
                for halfp in range(2):
                    off = halfp * 1024
                    for c in range(DC):
                        base = c * T + off
                        xc = lnwp.tile([128, 1024], bf16, tag=f"lnxc{halfp}")
                        nc.vector.tensor_tensor(out=xc[:],
                                                in0=xt[:, base:base + 1024],
                                                in1=mean_b[:, off:off + 1024],
                                                op=ALU.subtract)
                        nc.vector.tensor_tensor(out=xn[:, base:base + 1024],
                                                in0=xc[:],
                                                in1=rstd_b[:, off:off + 1024],
                                                op=ALU.mult)
                    if halfp == 0:
                        qk_proj_part(2, 0)  # k pair0, keys 0:1024
                        qk_proj_part(0, 0)  # q pair0, tokens 0:1024

            # ============ Phase 2: attention + output projection ============
            with tc.tile_pool(name="attn", bufs=1) as attnp:
                eblk0 = attnp.tile([128, KC * 1024], bf16, tag="eblk0")
                eblk1 = attnp.tile([128, KC * 1024], bf16, tag="eblk1")

                # ---- filler units (~1-4us of PE work each, FIFO on psD) ----
                def make_qk_units(m, tbp):
                    def u0():
                        slot = ps("psD")
                        qk_mms(m, tbp, 0, slot)
                        qk_mms(m, tbp, 1, slot)
                        qk_bias(m, tbp, slot)

                    return [(3900, u0)]

                def make_v_units(tq):
                    h = {}

                    def mk(half):
                        def u():
                            if half == 0:
                                h["slot"] = ps("psD")
                            o = h["slot"][:, half * 256:(half + 1) * 256]
                            tt = tq * 4 + half
                            nc.tensor.matmul(o, onesrow[0:1, 0:128],
                                             bqkv[0:1, 512:768],
                                             start=True, stop=False)
                            for c in range(DC):
                                nc.tensor.matmul(
                                    o,
                                    xn[:, c * T + tt * 128:c * T + (tt + 1) * 128],
                                    wqkv[:, c * F + 512:c * F + 768],
                                    start=False, stop=(c == DC - 1))
                        return u

                    def ucopy():
                        nc.vector.tensor_copy(
                            out=vsb_rr[:, tq * 4:(tq + 1) * 4, :, 64:192],
                            in_=h["slot"][:].rearrange(
                                "p (q hp y) -> p q hp y", q=4, hp=2, y=128))

                    return [(1250, mk(0)), (1250, mk(1)), (1250, mk(2)),
                            (1250, mk(3)), (60, ucopy)]

                def normalize(blk):
                    qb, pair, av = blk
                    # even head: den row 0 cols 0:512; odd: den row 127 cols
                    # 512:1024 -> hop to row 0, then one broadcast
                    # reciprocal_approx_fast only works at base partition 0,
                    # so hop the odd head's raw den from row 96 to row 0 first
                    rc = workp.tile([128, 1024], fp32, tag="recf")
                    nc.vector.tensor_copy(out=rc[96:97, 0:512],
                                          in_=av[96:97, 512:1024])
                    denrow = workp.tile([1, 512], fp32, tag="denrow")
                    nc.sync.dma_start(denrow[0:1, :], rc[96:97, 0:512])
                    nc.vector.reciprocal_approx_fast(
                        out=rc[0:1, 0:512], in_=av[0:1, 0:512])
                    nc.vector.reciprocal_approx_fast(
                        out=rc[0:1, 512:1024], in_=denrow[0:1, :])
                    rcb = workp.tile([1, 1024], bf16, tag="recb")
                    nc.vector.tensor_copy(out=rcb[0:1, :], in_=rc[0:1, :])
                    rbc = workp.tile([128, 1024], bf16, tag="rbcs")
                    nc.gpsimd.partition_broadcast(rbc[:], rcb[0:1, :])
                    un = workp.tile([128, 1024], bf16, tag="avun")
                    nc.vector.tensor_copy(out=un[64:128, 0:512],
                                          in_=av[64:128, 0:512])
                    nc.vector.tensor_copy(out=un[0:64, 512:1024],
                                          in_=av[0:64, 512:1024])
                    dsl = slice(pair * T + qb * QW, pair * T + (qb + 1) * QW)
                    nc.vector.tensor_tensor(out=aot[64:128, dsl],
                                            in0=un[64:128, 0:512],
                                            in1=rbc[64:128, 0:512],
                                            op=ALU.mult)
                    nc.vector.tensor_tensor(out=aot[0:64, dsl],
                                            in0=un[0:64, 512:1024],
                                            in1=rbc[0:64, 512:1024],
                                            op=ALU.mult)

                def outproj_grp(qb, mp, tag="psD"):
                    qsl = slice(qb * QW, (qb + 1) * QW)
                    slot = ps(tag)
                    for half in range(2):
                        m = 2 * mp + half
                        o = slot[:, half * 512:(half + 1) * 512]
                        for c2 in range(2):
                            nc.tensor.matmul(
                                o,
                                wout[:, c2 * D + m * 128:c2 * D + (m + 1) * 128],
                                aot[:, c2 * T + qb * QW:c2 * T + (qb + 1) * QW],
                                start=(c2 == 0), stop=(c2 == 1))
                    ob = workp.tile([128, 1024], fp32, tag="ob")
                    for half in range(2):
                        m = 2 * mp + half
                        nc.vector.tensor_scalar(
                            out=ob[:, half * 512:(half + 1) * 512],
                            in0=slot[:, half * 512:(half + 1) * 512],
                            scalar1=bout[:, m:m + 1], scalar2=None,
                            op0=ALU.add)
                    for half in range(2):
                        m = 2 * mp + half
                        nc.sync.dma_start(
                            out_d[m * 128:(m + 1) * 128, qsl],
                            ob[:, half * 512:(half + 1) * 512])

                # startup fillers, ordered by deadline (E stream needs qk
                # parts; AV needs v parts; outproj comes much later).
                # Entries are (cost_ns, fn, label); units of one psum-slot
                # group stay contiguous (FIFO) so psD is never interleaved.
                fillers = []

                def addg(units, label):
                    fillers.extend((c, fn, label) for c, fn in units)

                addg(make_qk_units(1, 0), "qk_1_0")  # q pair1 toks 0:1024
                addg(make_qk_units(1, 1), "qk_1_1")  # q pair1 toks 1024:2048
                addg(make_qk_units(2, 1), "qk_2_1")  # k pair0 keys 1024:2048
                addg(make_qk_units(0, 1), "qk_0_1")  # q pair0 toks 1024:2048
                addg(make_v_units(0), "v_0")
                addg(make_v_units(1), "v_1")
                addg(make_qk_units(3, 0), "qk_3_0")  # k pair1 keys 0:1024
                addg(make_v_units(2), "v_2")
                addg(make_qk_units(3, 1), "qk_3_1")  # k pair1 keys 1024:2048
                addg(make_v_units(3), "v_3")

                # build-time PE-debt model: pops fillers only when the PE has
                # slack vs the exp pace, so E matmuls never starve the ACT.
                # Hard deadlines (force_through) guarantee producers are
                # emitted before their consumers regardless of the model.
                ACT_NS = 1150.0 * (KC - len(SCHR_CHUNKS)) / KC
                E_NS, AV_NS = 390.0, 460.0
                CAP = 2400.0
                debt = [0.0]

                def pop1():
                    cost, fn, lab = fillers.pop(0)
                    fn()
                    debt[0] += cost

                def tick(fixed):
                    debt[0] = max(debt[0] + fixed - ACT_NS, 0.0)
                    while fillers and debt[0] + fillers[0][0] <= CAP:
                        pop1()

                def force_through(label):
                    while any(e[2] == label for e in fillers):
                        pop1()

                block_order = [(0, 0), (1, 0), (2, 0), (3, 0),
                               (0, 1), (1, 1), (2, 1), (3, 1)]
                prev = None
                for bi, (qb, pair) in enumerate(block_order):
                    qsl = slice(qb * QW, (qb + 1) * QW)
                    eblk = (eblk0, eblk1)[bi % 2]
                    qm = qk[:, (0 + pair) * T:(1 + pair) * T]
                    km = qk[:, (2 + pair) * T:(3 + pair) * T]
                    # hard deadlines: qk parts this block reads, plus the
                    # pending normalize (frees psC for this block's av)
                    force_through("norm")
                    force_through(f"qk_{2 + pair}_0")
                    force_through(f"qk_{pair}_{0 if qb < 2 else 1}")
                    if prev is not None:
                        pqb, ppair, peblk = prev
                        pav = ps("psC")

                        def av_mms(c, av=pav, pair=ppair, eblk=peblk):
                            for h in range(2):
                                hh = pair * 2 + h
                                nc.tensor.matmul(
                                    av[:, h * 512:(h + 1) * 512],
                                    vsb[:, (c * NH + hh) * 128:(c * NH + hh + 1) * 128],
                                    eblk[:, c * 1024 + h * 512:c * 1024 + (h + 1) * 512],
                                    start=(c == 0), stop=(c == KC - 1))
                    for c in range(KC):
                        if c == 7:
                            force_through(f"qk_{2 + pair}_1")
                        if prev is not None and c % 4 == 0:
                            force_through(f"v_{c // 4}")
                        eps_ = ps_ab()
                        ksl = slice(c * 128, (c + 1) * 128)
                        nc.tensor.matmul(eps_[:, 0:512], km[0:64, ksl],
                                         qm[0:64, qsl],
                                         tile_position=(0, 0))
                        nc.tensor.matmul(eps_[:, 512:1024], km[64:128, ksl],
                                         qm[64:128, qsl],
                                         tile_position=(64, 0))
                        if c in SCHR_CHUNKS:
                            # Schraudolph exp on the DVE: linear-interp 2^x via
                            # int16 bf16-bit-pattern; softmax absorbs the bias
                            nc.vector.tensor_scalar(
                                out=eblk[:, c * 1024:(c + 1) * 1024].bitcast(i16),
                                in0=eps_[:], scalar1=SCHR_A, scalar2=SCHR_B,
                                op0=ALU.mult, op1=ALU.add)
                        else:
                            nc.scalar.activation(
                                eblk[:, c * 1024:(c + 1) * 1024], eps_[:],
                                AF.Exp)
                        if prev is not None:
                            av_mms(c)
                            tick(E_NS + AV_NS)
                        else:
                            tick(E_NS)
                    if prev is not None:
                        blk = (pqb, ppair, pav)
                        fillers.insert(0, (60, lambda blk=blk: normalize(blk),
                                           "norm"))
                        if ppair == 1:
                            fillers.extend(
                                [(1050, lambda q=pqb, mp=mp: outproj_grp(q, mp),
                                  f"op_{pqb}") for mp in range(4)])
                    prev = (qb, pair, eblk)
                # tail: AV + normalize of the last block, remaining fillers
                pqb, ppair, peblk = prev
                pav = ps("psC")
                for c in range(KC):
                    for h in range(2):
                        hh = ppair * 2 + h
                        nc.tensor.matmul(
                            pav[:, h * 512:(h + 1) * 512],
                            vsb[:, (c * NH + hh) * 128:(c * NH + hh + 1) * 128],
                            peblk[:, c * 1024 + h * 512:c * 1024 + (h + 1) * 512],
                            start=(c == 0), stop=(c == KC - 1))
                    if c % 3 == 2 and fillers:
                        fillers.pop(0)[1]()
                while fillers:
                    fillers.pop(0)[1]()
                normalize((pqb, ppair, pav))  # noqa: the last block's own
                for mp in range(4):
                    outproj_grp(pqb, mp, tag=["psA", "psB"][mp % 2])
                if dbg:
                    nc.sync.dma_start(dbg["xn"][:], xn[:])
                    nc.sync.dma_start(dbg["qk"][:], qk[:])
                    nc.sync.dma_start(dbg["vsb"][:], vsb[:])
                    nc.sync.dma_start(dbg["aot"][:], aot[:])

    nc.compile()
    top = 0
    for alloc in nc.m.functions[0].allocations:
        try:
            mls = alloc.memorylocations
        except Exception:
            continue
        for ml in mls:
            if "SB" not in str(ml.type):
                continue
            a = ml.addr() if callable(ml.addr) else ml.addr
            s = ml.size() if callable(ml.size) else ml.size
            d = ml.dims() if callable(ml.dims) else ml.dims
            nparts = max(int(d[0]), 1) if d else 128
            top = max(top, int(a) + int(s) // nparts)
    assert top <= 208 * 1024, (
        f"SBUF envelope {top} B/partition exceeds usable 212992 "
        f"(writes past it corrupt the bass reserve)")
    return nc


def _prep_inputs(x, gamma, beta, w_qkv, w_out, b_out):
    import ml_dtypes

    bf16 = ml_dtypes.bfloat16
    wg = (w_qkv * gamma[:, None]).astype(np.float32)  # fold gamma
    bias_full = (beta @ w_qkv).astype(np.float32)  # fold beta
    in_maps = []
    for core in range(NCORES):
        b, g = divmod(core, 4)
        cs = slice(g * 256, (g + 1) * 256)
        qc = wg[:, 0 * D:1 * D][:, cs] * SCALE
        kc = wg[:, 1 * D:2 * D][:, cs]
        vc = wg[:, 2 * D:3 * D][:, cs]
        w_core = np.concatenate([qc, kc, vc], axis=1)  # [1024, 768]
        bq = bias_full[0 * D:1 * D][cs] * SCALE
        bk = bias_full[1 * D:2 * D][cs]
        bv = bias_full[2 * D:3 * D][cs]
        b_core = np.concatenate([bq, bk, bv])[None, :]  # [1, 768]
        xt = np.ascontiguousarray(x[b].T)  # [1024, 2048]
        xt_sb = xt.reshape(DC, 128, T).transpose(1, 0, 2).reshape(128, DC * T)
        wqkv_sb = w_core.reshape(DC, 128, F).transpose(1, 0, 2).reshape(128, DC * F)
        wout_core = w_out[g * 256:(g + 1) * 256, :]  # [256, 1024]
        # dense aot layout: chunk p rows 64:128 = even head 2p, rows 0:64 =
        # odd head 2p+1
        wout_sb = np.zeros((128, 2 * D), np.float32)
        for p in range(2):
            wout_sb[64:128, p * D:(p + 1) * D] = wout_core[(2 * p) * 64:(2 * p + 1) * 64]
            wout_sb[0:64, p * D:(p + 1) * D] = wout_core[(2 * p + 1) * 64:(2 * p + 2) * 64]
        in_maps.append({
            "xt": np.ascontiguousarray(xt_sb).astype(bf16),
            "wqkv": np.ascontiguousarray(wqkv_sb).astype(bf16),
            "wout": np.ascontiguousarray(wout_sb).astype(bf16),
            "bqkv": np.ascontiguousarray(b_core).astype(bf16),
            "bqkc": np.ascontiguousarray(b_core[0, :512].reshape(4, 128).T).astype(np.float32),
            "bout": np.ascontiguousarray(b_out.reshape(8, 128).T).astype(np.float32),
            "seed": np.zeros((1, _src_tag()), np.float32),
        })
    return in_maps


def kernel(x, gamma, beta, w_qkv, w_out, b_out, _want_trace=False):
    from concourse.bass_utils import run_bass_kernel_spmd

    x = np.asarray(x, dtype=np.float32)
    gamma = np.asarray(gamma, dtype=np.float32)
    beta = np.asarray(beta, dtype=np.float32)
    w_qkv = np.asarray(w_qkv, dtype=np.float32)
    w_out = np.asarray(w_out, dtype=np.float32)
    b_out = np.asarray(b_out, dtype=np.float32)

    if "nc" not in _cache:
        _cache["nc"] = _build()
    nc = _cache["nc"]
    in_maps = _prep_inputs(x, gamma, beta, w_qkv, w_out, b_out)
    res = run_bass_kernel_spmd(nc, in_maps, core_ids=list(range(NCORES)),
                               trace=_want_trace)
    _cache["last_result"] = res
    out = np.empty((B, S, D), dtype=np.float32)
    for b in range(B):
        acc = np.zeros((D, T), dtype=np.float32)
        for g in range(4):
            acc += res.results[b * 4 + g]["out"]
        out[b] = acc.T
    return out


# revision 19
# speedup vs baseline: 1.0220x; 1.0220x over previous
"""Fused LayerNorm + multi-head attention + output projection on 8 TRN2 cores.

Sharding: core c handles batch b = c//4 and head group g = c%4 (4 of 16 heads).
Each core computes LN(x[b]) (replicated within the batch's 4 cores), the qkv
projection for its heads, attention, and a partial output projection (w_out
rows for its heads). The host sums the 4 partials per batch.

On-chip layout is fully transposed ([feature, token]); the host pre-transposes
x, folds gamma / softmax scale / beta into the weights, and packs everything in
SBUF-ready layouts, so the kernel needs zero on-chip transposes:

  xn^T   [D, T]   = LayerNorm(x)^T        (stats via ones-matmul broadcast)
  q^T/k^T [dh, T] = W_q/k^T-slices @ xn^T (feature-major)
  v      [T, dh]  = xn^T-tiles.T @ W_v    (token-major, swapped operands)
  E^T    [k, q]   = exp(K Q^T)            (no max subtraction: scores ~N(0,1))
  av^T   [dh, q]  = V-block @ E^T         (denominator rows ride along in M)
  out^T  [D, T]   = w_out-slice^T @ (av^T/den)

V-block layout packs head pairs densely: even head -> [den@0 | 0*63 | V@64:128],
odd head -> [V@0:64 | zeros | den@96], so the attention output lands dense in a
2-chunk aot and the output projection contracts over 2 chunks instead of 4.
The odd head's denominator is hopped from partition 96 to 0 via a tiny DMA
before the partition broadcast (broadcast must source partition 0).

NOTE: keep the per-partition SBUF envelope under 208 KiB (asserted post-
compile) — allocations past it land in the bass reserve and corrupt
framework state at runtime in ways that show up as garbage tiles.
"""

import numpy as np

HEADS = 16
DIM_HEAD = 64
SCALE = DIM_HEAD**-0.5
EPS = 1e-5
B, S, D = 2, 2048, 1024
T = S
NCORES = 8
NH = 4  # heads per core
F = 3 * NH * DIM_HEAD  # 768 features per core: [q(256) | k(256) | v(256)]
DC = D // 128  # 8 contraction chunks
KC = T // 128  # 16 key chunks
QB = 4  # q blocks
QW = T // QB  # 512 q block width

_cache = {}


def _src_tag():
    # Cache-buster: the jit/NEFF cache can key on the HLO signature without
    # the embedded kernel program; giving the module an input whose shape
    # depends on this file's content forces a recompile when the kernel
    # changes.
    import hashlib
    with open(__file__, "rb") as f:
        return int(hashlib.sha256(f.read()).hexdigest()[:8], 16) % 120 + 8


def _build():
    import concourse.bacc as bacc
    import concourse.mybir as mybir
    import concourse.tile as tile

    fp32 = mybir.dt.float32
    bf16 = mybir.dt.bfloat16
    i16 = mybir.dt.int16
    AF = mybir.ActivationFunctionType
    ALU = mybir.AluOpType
    # Schraudolph exp offload to the DVE for these key-chunks of each block:
    # round(s*128/ln2 + B) as int16, bitcast bf16 == exp(s)*(1+-3%); the
    # softmax denominator mixes engines, so B centers the log error to keep
    # the approximation unbiased vs the exact ACT exps.
    SCHR_CHUNKS = _cache.get("schr_chunks", frozenset({2, 4, 7, 9, 12, 14}))
    SCHR_A = 128.0 / float(np.log(2.0))
    SCHR_B = 127.0 * 128.0 - 7.34

    nc = bacc.Bacc("TRN2", target_bir_lowering=False, debug=False,
                   num_devices=NCORES)
    xt_d = nc.declare_dram_parameter("xt", [128, DC * T], bf16, isOutput=False)
    wqkv_d = nc.declare_dram_parameter("wqkv", [128, DC * F], bf16, isOutput=False)
    wout_d = nc.declare_dram_parameter("wout", [128, 2 * D], bf16, isOutput=False)
    bqkv_d = nc.declare_dram_parameter("bqkv", [1, F], bf16, isOutput=False)
    bqkc_d = nc.declare_dram_parameter("bqkc", [128, 4], fp32, isOutput=False)
    bout_d = nc.declare_dram_parameter("bout", [128, 8], fp32, isOutput=False)
    seed_d = nc.declare_dram_parameter("seed", [1, _src_tag()], fp32,
                                       isOutput=False)
    out_d = nc.declare_dram_parameter("out", [D, T], fp32, isOutput=True)
    dbg = {}
    if _cache.get("debug"):
        dbg["xn"] = nc.declare_dram_parameter("dbg_xn", [128, DC * T], bf16, isOutput=True)
        dbg["qk"] = nc.declare_dram_parameter("dbg_qk", [128, 4 * T], bf16, isOutput=True)
        dbg["vsb"] = nc.declare_dram_parameter("dbg_vsb", [128, KC * NH * 128], bf16, isOutput=True)
        dbg["aot"] = nc.declare_dram_parameter("dbg_aot", [128, 2 * T], bf16, isOutput=True)

    with tile.TileContext(nc) as tc:
        with (
            tc.tile_pool(name="const", bufs=1) as constp,
            tc.tile_pool(name="big", bufs=1) as bigp,
            tc.tile_pool(name="work", bufs=2) as workp,
            tc.tile_pool(name="psum", bufs=1, space="PSUM") as psump,
        ):
            # ---- persistent SBUF ----
            seedt = constp.tile([1, 128], fp32, tag="seedt")
            nc.sync.dma_start(seedt[0:1, 0:seed_d.shape[1]], seed_d[:])
            ones128 = constp.tile([128, 128], bf16, tag="ones128")
            nc.gpsimd.memset(ones128[:], 1.0)
            onesrow = constp.tile([1, QW], bf16, tag="onesrow")
            nc.gpsimd.memset(onesrow[:], 1.0)
            wqkv = constp.tile([128, DC * F], bf16, tag="wqkv")
            wout = constp.tile([128, 2 * D], bf16, tag="wout")
            bqkv = constp.tile([1, F], bf16, tag="bqkv")
            bqkc = constp.tile([128, 4], fp32, tag="bqkc")
            bout = constp.tile([128, 8], fp32, tag="bout")

            xn = bigp.tile([128, DC * T], bf16, tag="xn")  # normalized x^T
            # q^T / k^T feature-major: m=0,1 -> q pairs 0,1; m=2,3 -> k
            qk = bigp.tile([128, 4 * T], bf16, tag="qk")
            # v blocks, 128 wide per (k-chunk, head):
            #   even head: [den(1) | zeros(63) | V(64)]  (V at rows 64:128)
            #   odd head:  [V(64) | zeros | den@96 | zeros]  (V at rows 0:64)
            vsb = bigp.tile([128, KC * NH * 128], bf16, tag="vsb")
            nc.gpsimd.memset(vsb[:], 0.0)
            vsb_rr = vsb[:].rearrange("p (c hp x) -> p c hp x", hp=2, x=256)
            nc.gpsimd.memset(vsb_rr[:, :, :, 0:1], 1.0)      # even-head den col
            nc.gpsimd.memset(vsb_rr[:, :, :, 224:225], 1.0)  # odd-head den col (row 96)
            # attention output^T, dense: chunk p = head pair p,
            # rows 64:128 = even head 2p, rows 0:64 = odd head 2p+1
            aot = bigp.tile([128, 2 * T], bf16, tag="aot")

            # psum slots: 4 tags x [128, 1024] (2 banks each) = 8 banks
            ps_n = [0]

            def ps(tag):
                ps_n[0] += 1
                return psump.tile([128, 1024], fp32, tag=tag,
                                  name=f"ps_{tag}_{ps_n[0]}")

            ab = [0]

            def ps_ab():
                ab[0] += 1
                return ps(["psA", "psB"][ab[0] % 2])

            def qk_mms(m, tbp, half, slot):
                tb = tbp * 2 + half
                o = slot[:, half * 512:(half + 1) * 512]
                for c in range(DC):
                    nc.tensor.matmul(
                        o,
                        wqkv[:, c * F + m * 128:c * F + (m + 1) * 128],
                        xn[:, c * T + tb * 512:c * T + (tb + 1) * 512],
                        start=(c == 0), stop=(c == DC - 1))

            def qk_bias(m, tbp, slot):
                nc.vector.tensor_scalar(
                    out=qk[:, m * T + tbp * 1024:m * T + (tbp + 1) * 1024],
                    in0=slot[:], scalar1=bqkc[:, m:m + 1], scalar2=None,
                    op0=ALU.add)

            def qk_proj_part(m, tbp):
                slot = ps_ab()
                qk_mms(m, tbp, 0, slot)
                qk_mms(m, tbp, 1, slot)
                qk_bias(m, tbp, slot)

            # ================= Phase 1: LayerNorm =================
            with (tc.tile_pool(name="ln", bufs=1) as lnp,
                  tc.tile_pool(name="lnw", bufs=2) as lnwp):
                xt = lnp.tile([128, DC * T], bf16, tag="xt")
                mean_b = lnp.tile([128, T], bf16, tag="mean_b")
                rstd_b = lnp.tile([128, T], bf16, tag="rstd_b")
                # x first: it gates everything. Weights after (needed later).
                for c in range(DC):
                    csl = slice(c * T, (c + 1) * T)
                    nc.sync.dma_start(xt[:, csl], xt_d[:, csl])
                nc.sync.dma_start(bqkc[:], bqkc_d[:])
                nc.sync.dma_start(wqkv[:], wqkv_d[:])
                nc.sync.dma_start(bqkv[:], bqkv_d[:])
                nc.sync.dma_start(bout[:], bout_d[:])
                nc.sync.dma_start(wout[:], wout_d[:])
                x2 = xn  # scratch: squares are consumed before xn is written
                for c in range(DC):
                    csl = slice(c * T, (c + 1) * T)
                    nc.vector.tensor_tensor(out=x2[:, csl], in0=xt[:, csl],
                                            in1=xt[:, csl], op=ALU.mult)
                # stats, chunk-outer so matmuls chase the DMAs; the x^2 pass
                # lags one chunk so the DVE stays ahead of the PE
                slots = [ps(t) for t in ("psA", "psB", "psC", "psD")]
                for c in range(DC + 1):
                    if c < DC:
                        for tb in range(4):
                            sl = slice(c * T + tb * 512, c * T + (tb + 1) * 512)
                            nc.tensor.matmul(slots[tb][:, 0:512], ones128[:],
                                             xt[:, sl],
                                             start=(c == 0), stop=(c == DC - 1))
                    if c > 0:
                        cm = c - 1
                        for tb in range(4):
                            sl = slice(cm * T + tb * 512, cm * T + (tb + 1) * 512)
                            nc.tensor.matmul(slots[tb][:, 512:1024], ones128[:],
                                             x2[:, sl],
                                             start=(cm == 0), stop=(cm == DC - 1))
                def tb_post(tb):
                    s_ps, q_ps = slots[tb][:, 0:512], slots[tb][:, 512:1024]
                    tsl = slice(tb * 512, (tb + 1) * 512)
                    nc.vector.tensor_scalar(out=mean_b[:, tsl], in0=s_ps,
                                            scalar1=1.0 / D, scalar2=None,
                                            op0=ALU.mult)
                    t1 = lnwp.tile([128, 512], fp32, tag="lnt1")
                    nc.vector.tensor_scalar(out=t1[:], in0=q_ps,
                                            scalar1=1.0 / D, scalar2=EPS,
                                            op0=ALU.mult, op1=ALU.add)
                    m2 = lnwp.tile([128, 512], fp32, tag="lnm2")
                    nc.vector.tensor_tensor(out=m2[:], in0=mean_b[:, tsl],
                                            in1=mean_b[:, tsl], op=ALU.mult)
                    var = lnwp.tile([128, 512], fp32, tag="lnvar")
                    nc.vector.tensor_tensor(out=var[:], in0=t1[:], in1=m2[:],
                                            op=ALU.subtract)
                    rcv = lnwp.tile([128, 512], fp32, tag="lnrcv")
                    nc.vector.reciprocal_approx_fast(out=rcv[:], in_=var[:])
                    nc.scalar.activation(rstd_b[:, tsl], rcv[:], AF.Sqrt)
                # normalize in half-token panels: each half's stats postproc
                # runs just before it so the DVE reaches xn half0 ASAP; q/k
                # projections for the first half start while the second half
                # is still on the DVE
                for halfp in range(2):
                    tb_post(2 * halfp)
                    tb_post(2 * halfp + 1)
                    if halfp == 0:
                        # preload the exp table set while ACT is idle, so the
                        # first real exp doesn't pay the ~1.3us table load
                        dume = lnwp.tile([1, 8], fp32, tag="dume")
                        nc.scalar.activation(dume[:], rstd_b[0:1, 0:8], AF.Exp)
                    off = halfp * 1024
                    for c in range(DC):
                        base = c * T + off
                        xc = lnwp.tile([128, 1024], bf16, tag=f"lnxc{halfp}")
                        nc.vector.tensor_tensor(out=xc[:],
                                                in0=xt[:, base:base + 1024],
                                                in1=mean_b[:, off:off + 1024],
                                                op=ALU.subtract)
                        nc.vector.tensor_tensor(out=xn[:, base:base + 1024],
                                                in0=xc[:],
                                                in1=rstd_b[:, off:off + 1024],
                                                op=ALU.mult)
                    if halfp == 0:
                        qk_proj_part(2, 0)  # k pair0, keys 0:1024
                        qk_proj_part(0, 0)  # q pair0, tokens 0:1024

            # ============ Phase 2: attention + output projection ============
            with tc.tile_pool(name="attn", bufs=1) as attnp:
                eblk0 = attnp.tile([128, KC * 1024], bf16, tag="eblk0")
                eblk1 = attnp.tile([128, KC * 1024], bf16, tag="eblk1")

                # ---- filler units (~1-4us of PE work each, FIFO on psD) ----
                def make_qk_units(m, tbp):
                    def u0():
                        slot = ps("psD")
                        qk_mms(m, tbp, 0, slot)
                        qk_mms(m, tbp, 1, slot)
                        qk_bias(m, tbp, slot)

                    return [(3900, u0)]

                def make_v_units(tq):
                    h = {}

                    def mk(half):
                        def u():
                            if half == 0:
                                h["slot"] = ps("psD")
                            o = h["slot"][:, half * 256:(half + 1) * 256]
                            tt = tq * 4 + half
                            nc.tensor.matmul(o, onesrow[0:1, 0:128],
                                             bqkv[0:1, 512:768],
                                             start=True, stop=False)
                            for c in range(DC):
                                nc.tensor.matmul(
                                    o,
                                    xn[:, c * T + tt * 128:c * T + (tt + 1) * 128],
                                    wqkv[:, c * F + 512:c * F + 768],
                                    start=False, stop=(c == DC - 1))
                        return u

                    def ucopy():
                        nc.vector.tensor_copy(
                            out=vsb_rr[:, tq * 4:(tq + 1) * 4, :, 64:192],
                            in_=h["slot"][:].rearrange(
                                "p (q hp y) -> p q hp y", q=4, hp=2, y=128))

                    return [(1250, mk(0)), (1250, mk(1)), (1250, mk(2)),
                            (1250, mk(3)), (60, ucopy)]

                def normalize(blk):
                    qb, pair, av = blk
                    # even head: den row 0 cols 0:512; odd: den row 127 cols
                    # 512:1024 -> hop to row 0, then one broadcast
                    # reciprocal_approx_fast only works at base partition 0,
                    # so hop the odd head's raw den from row 96 to row 0 first
                    rc = workp.tile([128, 1024], fp32, tag="recf")
                    nc.vector.tensor_copy(out=rc[96:97, 0:512],
                                          in_=av[96:97, 512:1024])
                    denrow = workp.tile([1, 512], fp32, tag="denrow")
                    nc.sync.dma_start(denrow[0:1, :], rc[96:97, 0:512])
                    nc.vector.reciprocal_approx_fast(
                        out=rc[0:1, 0:512], in_=av[0:1, 0:512])
                    nc.vector.reciprocal_approx_fast(
                        out=rc[0:1, 512:1024], in_=denrow[0:1, :])
                    rcb = workp.tile([1, 1024], bf16, tag="recb")
                    nc.vector.tensor_copy(out=rcb[0:1, :], in_=rc[0:1, :])
                    rbc = workp.tile([128, 1024], bf16, tag="rbcs")
                    nc.gpsimd.partition_broadcast(rbc[:], rcb[0:1, :])
                    un = workp.tile([128, 1024], bf16, tag="avun")
                    nc.vector.tensor_copy(out=un[64:128, 0:512],
                                          in_=av[64:128, 0:512])
                    nc.vector.tensor_copy(out=un[0:64, 512:1024],
                                          in_=av[0:64, 512:1024])
                    dsl = slice(pair * T + qb * QW, pair * T + (qb + 1) * QW)
                    nc.vector.tensor_tensor(out=aot[64:128, dsl],
                                            in0=un[64:128, 0:512],
                                            in1=rbc[64:128, 0:512],
                                            op=ALU.mult)
                    nc.vector.tensor_tensor(out=aot[0:64, dsl],
                                            in0=un[0:64, 512:1024],
                                            in1=rbc[0:64, 512:1024],
                                            op=ALU.mult)

                def outproj_grp(qb, mp, tag="psD"):
                    qsl = slice(qb * QW, (qb + 1) * QW)
                    slot = ps(tag)
                    for half in range(2):
                        m = 2 * mp + half
                        o = slot[:, half * 512:(half + 1) * 512]
                        for c2 in range(2):
                            nc.tensor.matmul(
                                o,
                                wout[:, c2 * D + m * 128:c2 * D + (m + 1) * 128],
                                aot[:, c2 * T + qb * QW:c2 * T + (qb + 1) * QW],
                                start=(c2 == 0), stop=(c2 == 1))
                    ob = workp.tile([128, 1024], fp32, tag="ob")
                    for half in range(2):
                        m = 2 * mp + half
                        nc.vector.tensor_scalar(
                            out=ob[:, half * 512:(half + 1) * 512],
                            in0=slot[:, half * 512:(half + 1) * 512],
                            scalar1=bout[:, m:m + 1], scalar2=None,
                            op0=ALU.add)
                    for half in range(2):
                        m = 2 * mp + half
                        nc.sync.dma_start(
                            out_d[m * 128:(m + 1) * 128, qsl],
                            ob[:, half * 512:(half + 1) * 512])

                # startup fillers, ordered by deadline (E stream needs qk
                # parts; AV needs v parts; outproj comes much later).
                # Entries are (cost_ns, fn, label); units of one psum-slot
                # group stay contiguous (FIFO) so psD is never interleaved.
                fillers = []

                def addg(units, label):
                    fillers.extend((c, fn, label) for c, fn in units)

                addg(make_qk_units(2, 1), "qk_2_1")  # k pair0 keys 1024:2048
                addg(make_qk_units(0, 1), "qk_0_1")  # q pair0 toks 1024:2048
                addg(make_v_units(0), "v_0")
                addg(make_v_units(1), "v_1")
                addg(make_qk_units(3, 0), "qk_3_0")  # k pair1 keys 0:1024
                addg(make_v_units(2), "v_2")
                addg(make_qk_units(3, 1), "qk_3_1")  # k pair1 keys 1024:2048
                addg(make_v_units(3), "v_3")
                addg(make_qk_units(1, 0), "qk_1_0")  # q pair1 toks 0:1024
                addg(make_qk_units(1, 1), "qk_1_1")  # q pair1 toks 1024:2048

                # build-time PE-debt model: pops fillers only when the PE has
                # slack vs the exp pace, so E matmuls never starve the ACT.
                # Hard deadlines (force_through) guarantee producers are
                # emitted before their consumers regardless of the model.
                ACT_NS = (1150.0 * (KC - len(SCHR_CHUNKS))
                          + 800.0 * len(SCHR_CHUNKS)) / KC
                E_NS, AV_NS = 390.0, 460.0
                CAP = 3000.0
                debt = [0.0]

                def insert_priority(items):
                    # never split the front psum-slot group
                    i = 0
                    if fillers:
                        lab0 = fillers[0][2]
                        while i < len(fillers) and fillers[i][2] == lab0:
                            i += 1
                    fillers[i:i] = items

                def pop1():
                    cost, fn, lab = fillers.pop(0)
                    fn()
                    debt[0] += cost

                def tick(fixed):
                    debt[0] = max(debt[0] + fixed - ACT_NS, 0.0)
                    while fillers and debt[0] + fillers[0][0] <= CAP:
                        pop1()

                def force_through(label):
                    while any(e[2] == label for e in fillers):
                        pop1()

                block_order = [(0, 0), (1, 0), (2, 0), (3, 0),
                               (0, 1), (1, 1), (2, 1), (3, 1)]
                prev = None
                for bi, (qb, pair) in enumerate(block_order):
                    qsl = slice(qb * QW, (qb + 1) * QW)
                    eblk = (eblk0, eblk1)[bi % 2]
                    qm = qk[:, (0 + pair) * T:(1 + pair) * T]
                    km = qk[:, (2 + pair) * T:(3 + pair) * T]
                    # hard deadlines: qk parts this block reads, plus the
                    # pending normalize (frees psC for this block's av)
                    force_through("norm")
                    force_through(f"qk_{2 + pair}_0")
                    force_through(f"qk_{pair}_{0 if qb < 2 else 1}")
                    if prev is not None:
                        pqb, ppair, peblk = prev
                        pav = ps("psC")

                        def av_mms(c, av=pav, pair=ppair, eblk=peblk):
                            for h in range(2):
                                hh = pair * 2 + h
                                nc.tensor.matmul(
                                    av[:, h * 512:(h + 1) * 512],
                                    vsb[:, (c * NH + hh) * 128:(c * NH + hh + 1) * 128],
                                    eblk[:, c * 1024 + h * 512:c * 1024 + (h + 1) * 512],
                                    start=(c == 0), stop=(c == KC - 1))
                    for c in range(KC):
                        if c == 7:
                            force_through(f"qk_{2 + pair}_1")
                        if prev is not None and c % 4 == 0:
                            force_through(f"v_{c // 4}")
                        eps_ = ps_ab()
                        ksl = slice(c * 128, (c + 1) * 128)
                        nc.tensor.matmul(eps_[:, 0:512], km[0:64, ksl],
                                         qm[0:64, qsl],
                                         tile_position=(0, 0))
                        nc.tensor.matmul(eps_[:, 512:1024], km[64:128, ksl],
                                         qm[64:128, qsl],
                                         tile_position=(64, 0))
                        if c in SCHR_CHUNKS:
                            # Schraudolph exp on the DVE: linear-interp 2^x via
                            # int16 bf16-bit-pattern; softmax absorbs the bias
                            nc.vector.tensor_scalar(
                                out=eblk[:, c * 1024:(c + 1) * 1024].bitcast(i16),
                                in0=eps_[:], scalar1=SCHR_A, scalar2=SCHR_B,
                                op0=ALU.mult, op1=ALU.add)
                        else:
                            nc.scalar.activation(
                                eblk[:, c * 1024:(c + 1) * 1024], eps_[:],
                                AF.Exp)
                        if prev is not None:
                            av_mms(c)
                            tick(E_NS + AV_NS)
                        else:
                            tick(E_NS)
                    if prev is not None:
                        blk = (pqb, ppair, pav)
                        items = [(60, lambda blk=blk: normalize(blk), "norm")]
                        if ppair == 1:
                            items += [(1050,
                                       lambda q=pqb, mp=mp: outproj_grp(q, mp),
                                       f"op_{pqb}") for mp in range(4)]
                        insert_priority(items)
                    prev = (qb, pair, eblk)
                # tail: AV + normalize of the last block, remaining fillers
                pqb, ppair, peblk = prev
                pav = ps("psC")
                for c in range(KC):
                    for h in range(2):
                        hh = ppair * 2 + h
                        nc.tensor.matmul(
                            pav[:, h * 512:(h + 1) * 512],
                            vsb[:, (c * NH + hh) * 128:(c * NH + hh + 1) * 128],
                            peblk[:, c * 1024 + h * 512:c * 1024 + (h + 1) * 512],
                            start=(c == 0), stop=(c == KC - 1))
                    if fillers:
                        fillers.pop(0)[1]()
                while fillers:
                    fillers.pop(0)[1]()
                normalize((pqb, ppair, pav))  # noqa: the last block's own
                for mp in range(4):
                    outproj_grp(pqb, mp, tag=["psA", "psB"][mp % 2])
                if dbg:
                    nc.sync.dma_start(dbg["xn"][:], xn[:])
                    nc.sync.dma_start(dbg["qk"][:], qk[:])
                    nc.sync.dma_start(dbg["vsb"][:], vsb[:])
                    nc.sync.dma_start(dbg["aot"][:], aot[:])

    nc.compile()
    top = 0
    for alloc in nc.m.functions[0].allocations:
        try:
            mls = alloc.memorylocations
        except Exception:
            continue
        for ml in mls:
            if "SB" not in str(ml.type):
                continue
            a = ml.addr() if callable(ml.addr) else ml.addr
            s = ml.size() if callable(ml.size) else ml.size
            d = ml.dims() if callable(ml.dims) else ml.dims
            nparts = max(int(d[0]), 1) if d else 128
            top = max(top, int(a) + int(s) // nparts)
    assert top <= 208 * 1024, (
        f"SBUF envelope {top} B/partition exceeds usable 212992 "
        f"(writes past it corrupt the bass reserve)")
    return nc


def _prep_inputs(x, gamma, beta, w_qkv, w_out, b_out):
    import ml_dtypes

    bf16 = ml_dtypes.bfloat16
    wg = (w_qkv * gamma[:, None]).astype(np.float32)  # fold gamma
    bias_full = (beta @ w_qkv).astype(np.float32)  # fold beta
    in_maps = []
    for core in range(NCORES):
        b, g = divmod(core, 4)
        cs = slice(g * 256, (g + 1) * 256)
        qc = wg[:, 0 * D:1 * D][:, cs] * SCALE
        kc = wg[:, 1 * D:2 * D][:, cs]
        vc = wg[:, 2 * D:3 * D][:, cs]
        w_core = np.concatenate([qc, kc, vc], axis=1)  # [1024, 768]
        bq = bias_full[0 * D:1 * D][cs] * SCALE
        bk = bias_full[1 * D:2 * D][cs]
        bv = bias_full[2 * D:3 * D][cs]
        b_core = np.concatenate([bq, bk, bv])[None, :]  # [1, 768]
        xt = np.ascontiguousarray(x[b].T)  # [1024, 2048]
        xt_sb = xt.reshape(DC, 128, T).transpose(1, 0, 2).reshape(128, DC * T)
        wqkv_sb = w_core.reshape(DC, 128, F).transpose(1, 0, 2).reshape(128, DC * F)
        wout_core = w_out[g * 256:(g + 1) * 256, :]  # [256, 1024]
        # dense aot layout: chunk p rows 64:128 = even head 2p, rows 0:64 =
        # odd head 2p+1
        wout_sb = np.zeros((128, 2 * D), np.float32)
        for p in range(2):
            wout_sb[64:128, p * D:(p + 1) * D] = wout_core[(2 * p) * 64:(2 * p + 1) * 64]
            wout_sb[0:64, p * D:(p + 1) * D] = wout_core[(2 * p + 1) * 64:(2 * p + 2) * 64]
        in_maps.append({
            "xt": np.ascontiguousarray(xt_sb).astype(bf16),
            "wqkv": np.ascontiguousarray(wqkv_sb).astype(bf16),
            "wout": np.ascontiguousarray(wout_sb).astype(bf16),
            "bqkv": np.ascontiguousarray(b_core).astype(bf16),
            "bqkc": np.ascontiguousarray(b_core[0, :512].reshape(4, 128).T).astype(np.float32),
            "bout": np.ascontiguousarray(b_out.reshape(8, 128).T).astype(np.float32),
            "seed": np.zeros((1, _src_tag()), np.float32),
        })
    return in_maps


def kernel(x, gamma, beta, w_qkv, w_out, b_out, _want_trace=False):
    from concourse.bass_utils import run_bass_kernel_spmd

    x = np.asarray(x, dtype=np.float32)
    gamma = np.asarray(gamma, dtype=np.float32)
    beta = np.asarray(beta, dtype=np.float32)
    w_qkv = np.asarray(w_qkv, dtype=np.float32)
    w_out = np.asarray(w_out, dtype=np.float32)
    b_out = np.asarray(b_out, dtype=np.float32)

    if "nc" not in _cache:
        _cache["nc"] = _build()
    nc = _cache["nc"]
    in_maps = _prep_inputs(x, gamma, beta, w_qkv, w_out, b_out)
    res = run_bass_kernel_spmd(nc, in_maps, core_ids=list(range(NCORES)),
                               trace=_want_trace)
    _cache["last_result"] = res
    out = np.empty((B, S, D), dtype=np.float32)
    for b in range(B):
        acc = np.zeros((D, T), dtype=np.float32)
        for g in range(4):
            acc += res.results[b * 4 + g]["out"]
        out[b] = acc.T
    return out
